# revision 3
# baseline (speedup 1.0000x reference)
"""Trainium2 Bass kernel for MultiHeadMemAttn (mean-pooled-memory attention).

Full computation (per batch b):
    mem  = mean_pool(keyvalue, window=64, stride=64)          # [64, 512]
    hq   = query @ Wq.T ; hk = mem @ Wk.T ; hv = mem @ Wv.T   # heads=8, hd=64
    attn = softmax(hq @ hk.T / 8, over mem axis)
    out  = (attn @ hv) @ Wo.T

Sharding: data-parallel over batch across 8 cores (4 batches each),
weights replicated.  No collectives.

Device layout strategy (per core):
  - pooling is a PE matmul:   mem = P_c.T @ kv_chunk  (P_c slices of a
    host-built [128,126] band matrix, accumulated over 32 s-chunks in PSUM)
  - query tiles are PE-transposed to qT [d, i]; hqT = WqT.T-chunks @ qT
  - scores computed transposed: scoresT[j, i] = hkT_h.T @ hqT_h  (pairs of
    heads packed into one [128, 512] PSUM tile)
  - softmax without max-subtraction (|scores| <= ~6):  E = exp(s/8) on ACT;
    denominators via ones-matmul -> [2,512]; reciprocal on DVE; broadcast
    back over partitions via a tiny K=2 PE matmul; attn = E * bcast on DVE
  - uvecT[dv, i] = V_h.T-as-lhsT @ attn   (vecT comes out pre-transposed)
  - out[i, o] = vecT-chunks.T @ WoT  (natural layout, DMA straight out)
"""

import os
from contextlib import ExitStack

import numpy as np

import concourse.bass as bass
import concourse.mybir as mybir
import concourse.tile as tile
from concourse.bass_utils import run_bass_kernel_spmd

F32 = mybir.dt.float32

# ---------------------------------------------------------------------------
# Workaround: this walrus build only encodes ONE sem-wait per instruction
# ("Too many sync wait commands" in CoreV3GenImpl setupSyncWait), while
# Tile's sem-assignment freely attaches several.  Post-process the
# serialized BIR: move surplus waits onto injected same-engine NoOps placed
# immediately before the instruction (engine streams are in-order, so the
# NoOp chain stalls the engine exactly like multi-wait would).
import json as _json

_orig_to_json_bytes = bass.Bass.to_json_bytes


def _split_multi_waits(self, *args, **kwargs):
    raw = _orig_to_json_bytes(self, *args, **kwargs)
    d = _json.loads(raw)
    changed = False

    def fix_block(o):
        nonlocal changed
        if isinstance(o, dict):
            insts = o.get("instructions")
            if isinstance(insts, list):
                new = []
                for inst in insts:
                    si = inst.get("sync_info") if isinstance(inst, dict) else None
                    waits = (si or {}).get("on_wait") or []
                    if len(waits) > 1:
                        changed = True
                        for i, w in enumerate(waits[:-1]):
                            new.append(
                                {
                                    "name": f"{inst['name']}-sw{i}",
                                    "opcode": "NoOp",
                                    "engine": inst["engine"],
                                    "ins": [],
                                    "outs": [],
                                    "debug": inst.get("debug", 0),
                                    "sync_info": {
                                        "on_wait": [w],
                                        "on_update": [],
                                    },
                                }
                            )
                        si["on_wait"] = [waits[-1]]
                    new.append(inst)
                o["instructions"] = new
            for v in o.values():
                fix_block(v)
        elif isinstance(o, list):
            for v in o:
                fix_block(v)

    fix_block(d)
    if not changed:
        return raw
    return _json.dumps(d).encode()


bass.Bass.to_json_bytes = _split_multi_waits
# ---------------------------------------------------------------------------

NCORES = 8
B = 4          # batches per core
QLEN = 1024
S = 4096       # kv sequence length
D = 512        # hidden
H = 8          # heads
HD = 64        # head dim
MEM = 64       # mem_len (pooled length)
DC = D // 128  # 4 chunks of the hidden dim
ICN = 2        # i-chunks of 512 per batch
IT = 4         # 128-row tiles per i-chunk
KT = 8         # kv DMA tiles per batch (4 s-chunks of 128 each)

EXPF = mybir.ActivationFunctionType.Exp


def _build_nc() -> bass.Bass:
    nc = bass.Bass()
    q = nc.dram_tensor("query", [B, QLEN, D], F32, kind="ExternalInput")
    kv = nc.dram_tensor("keyvalue", [B, S, D], F32, kind="ExternalInput")
    wqT = nc.dram_tensor("wqT", [D, D], F32, kind="ExternalInput")
    wkT = nc.dram_tensor("wkT", [D, D], F32, kind="ExternalInput")
    wvT = nc.dram_tensor("wvT", [D, D], F32, kind="ExternalInput")
    woT = nc.dram_tensor("woT", [D, D], F32, kind="ExternalInput")
    poolD = nc.dram_tensor("poolD", [128, 126], F32, kind="ExternalInput")
    ident = nc.dram_tensor("ident", [128, 128], F32, kind="ExternalInput")
    ones2 = nc.dram_tensor("ones2", [128, 2], F32, kind="ExternalInput")
    expand2 = nc.dram_tensor("expand2", [2, 128], F32, kind="ExternalInput")
    out = nc.dram_tensor("out", [B, QLEN, D], F32, kind="ExternalOutput")

    # DRAM views for partition-major DMA
    q_v = q.rearrange("b (ic it p) d -> b ic p it d", ic=ICN, it=IT, p=128)
    kv_v = kv.rearrange("b (t c p) d -> b t p c d", t=KT, c=4, p=128)
    out_v = out.rearrange("b (ic it p) d -> b ic p it d", ic=ICN, it=IT, p=128)
    wq_v = wqT.rearrange("(dc p) o -> p dc o", p=128)
    wk_v = wkT.rearrange("(dc p) o -> p dc o", p=128)
    wv_v = wvT.rearrange("(dc p) o -> p dc o", p=128)
    wo_v = woT.rearrange("(dc p) o -> p dc o", p=128)

    with tile.TileContext(nc) as tc, ExitStack() as ctx:
        # SBUF pools
        singles = ctx.enter_context(tc.tile_pool(name="singles", bufs=1))
        kvp = ctx.enter_context(tc.tile_pool(name="kvp", bufs=2))
        qp = ctx.enter_context(tc.tile_pool(name="qp", bufs=2))
        qtp = ctx.enter_context(tc.tile_pool(name="qtp", bufs=2))
        hqp = ctx.enter_context(tc.tile_pool(name="hqp", bufs=2))
        memp = ctx.enter_context(tc.tile_pool(name="memp", bufs=2))
        ep = ctx.enter_context(tc.tile_pool(name="ep", bufs=3))
        attnp = ctx.enter_context(tc.tile_pool(name="attnp", bufs=3))
        rdp = ctx.enter_context(tc.tile_pool(name="rdp", bufs=3))
        vtp = ctx.enter_context(tc.tile_pool(name="vtp", bufs=2))
        outp = ctx.enter_context(tc.tile_pool(name="outp", bufs=2))
        # PSUM pools (8 banks total: 1 + 2 + 1 + 4)
        accp = ctx.enter_context(tc.tile_pool(name="accp", bufs=1, space="PSUM"))
        trp = ctx.enter_context(tc.tile_pool(name="trp", bufs=2, space="PSUM"))
        denp = ctx.enter_context(tc.tile_pool(name="denp", bufs=1, space="PSUM"))
        mmp = ctx.enter_context(tc.tile_pool(name="mmp", bufs=4, space="PSUM"))

        # one-time loads
        wq_sb = singles.tile([128, DC, D], F32)
        nc.sync.dma_start(out=wq_sb, in_=wq_v)
        wk_sb = singles.tile([128, DC, D], F32)
        nc.sync.dma_start(out=wk_sb, in_=wk_v)
        wv_sb = singles.tile([128, DC, D], F32)
        nc.sync.dma_start(out=wv_sb, in_=wv_v)
        wo_sb = singles.tile([128, DC, D], F32)
        nc.sync.dma_start(out=wo_sb, in_=wo_v)
        poolD_sb = singles.tile([128, 126], F32)
        nc.sync.dma_start(out=poolD_sb, in_=poolD[:, :])
        ident_sb = singles.tile([128, 128], F32)
        nc.sync.dma_start(out=ident_sb, in_=ident[:, :])
        ones2_sb = singles.tile([128, 2], F32)
        nc.sync.dma_start(out=ones2_sb, in_=ones2[:, :])
        expand2_sb = singles.tile([2, 128], F32)
        nc.sync.dma_start(out=expand2_sb, in_=expand2[:, :])

        for b in range(B):
            # ---- mean-pool keyvalue -> mem [64, 512] -------------------
            pacc = accp.tile([MEM, D], F32, tag="acc")
            for t in range(KT):
                kvt = kvp.tile([128, 4, D], F32, tag="kv")
                nc.sync.dma_start(out=kvt, in_=kv_v[b, t])
                for c in range(4):
                    sc = 4 * t + c
                    nc.tensor.matmul(
                        pacc,
                        lhsT=poolD_sb[:, 62 - 2 * sc : 126 - 2 * sc],
                        rhs=kvt[:, c, :],
                        start=(sc == 0),
                        stop=(sc == 31),
                    )
            mem_sb = memp.tile([MEM, D], F32, tag="mem")
            nc.scalar.copy(out=mem_sb, in_=pacc)

            # ---- memT [d, m] (4 chunks of 128 d) -----------------------
            trt = trp.tile([128, 4, MEM], F32, tag="tr")
            for c in range(4):
                nc.tensor.transpose(
                    trt[:, c, :],
                    mem_sb[:, 128 * c : 128 * (c + 1)],
                    ident_sb[0:MEM, 0:MEM],
                )
            memT_sb = memp.tile([128, 4, MEM], F32, tag="memT")
            nc.scalar.copy(out=memT_sb, in_=trt)

            # ---- hkT [o, m]  (4 chunks) --------------------------------
            hkT_sb = memp.tile([128, 4, MEM], F32, tag="hkT")
            for oc in range(4):
                hk_ps = mmp.tile([128, MEM], F32, tag="mm")
                for dc in range(DC):
                    nc.tensor.matmul(
                        hk_ps,
                        lhsT=wk_sb[:, dc, 128 * oc : 128 * (oc + 1)],
                        rhs=memT_sb[:, dc, :],
                        start=(dc == 0),
                        stop=(dc == DC - 1),
                    )
                nc.scalar.copy(out=hkT_sb[:, oc, :], in_=hk_ps)

            # ---- hv [m, o], duplicated into both partition halves ------
            hv_ps = mmp.tile([128, D], F32, tag="mm")
            for half in range(2):
                for dc in range(DC):
                    nc.tensor.matmul(
                        hv_ps[64 * half : 64 * half + 64, :],
                        lhsT=memT_sb[:, dc, :],
                        rhs=wv_sb[:, dc, :],
                        start=(dc == 0),
                        stop=(dc == DC - 1),
                        tile_position=(0, 64 * half),
                    )
            hv_sb = memp.tile([128, D], F32, tag="hv")
            nc.scalar.copy(out=hv_sb, in_=hv_ps)

            for ic in range(ICN):
                # ---- load q, transpose to qT [d, i] --------------------
                qt = qp.tile([128, IT, D], F32, tag="q")
                nc.sync.dma_start(out=qt, in_=q_v[b, ic])
                qT_sb = qtp.tile([128, DC, D], F32, tag="qT")
                for it in range(IT):
                    trq = trp.tile([128, 4, 128], F32, tag="tr")
                    for c in range(4):
                        nc.tensor.transpose(
                            trq[:, c, :],
                            qt[:, it, 128 * c : 128 * (c + 1)],
                            ident_sb,
                        )
                    nc.scalar.copy(
                        out=qT_sb[:, :, 128 * it : 128 * (it + 1)], in_=trq
                    )

                # ---- hqT [o, i] ----------------------------------------
                hqT_sb = hqp.tile([128, DC, D], F32, tag="hqT")
                for oc in range(DC):
                    hq_ps = mmp.tile([128, D], F32, tag="mm")
                    for dc in range(DC):
                        nc.tensor.matmul(
                            hq_ps,
                            lhsT=wq_sb[:, dc, 128 * oc : 128 * (oc + 1)],
                            rhs=qT_sb[:, dc, :],
                            start=(dc == 0),
                            stop=(dc == DC - 1),
                        )
                    nc.scalar.copy(out=hqT_sb[:, oc, :], in_=hq_ps)

                # ---- attention, one head-pair at a time ----------------
                vecT_sb = vtp.tile([128, 4, D], F32, tag="vecT")
                for p2 in range(4):
                    sc_ps = mmp.tile([128, D], F32, tag="mm")
                    for hh in range(2):
                        o0 = 64 * hh
                        nc.tensor.matmul(
                            sc_ps[o0 : o0 + 64, :],
                            lhsT=hkT_sb[o0 : o0 + 64, p2, :],
                            rhs=hqT_sb[o0 : o0 + 64, p2, :],
                            start=True,
                            stop=True,
                            tile_position=(o0, o0),
                        )
                    e_sb = ep.tile([128, D], F32, tag="e")
                    nc.scalar.activation(
                        out=e_sb, in_=sc_ps, func=EXPF, scale=0.125
                    )
                    den_ps = denp.tile([2, D], F32, tag="den")
                    nc.tensor.matmul(
                        den_ps, lhsT=ones2_sb, rhs=e_sb, start=True, stop=True
                    )
                    rden_sb = rdp.tile([2, D], F32, tag="rden")
                    nc.vector.reciprocal(out=rden_sb, in_=den_ps)
                    bc_ps = mmp.tile([128, D], F32, tag="mm")
                    nc.tensor.matmul(
                        bc_ps, lhsT=expand2_sb, rhs=rden_sb, start=True, stop=True
                    )
                    attn_sb = attnp.tile([128, D], F32, tag="attn")
                    nc.vector.tensor_mul(attn_sb, e_sb, bc_ps)
                    uv_ps = mmp.tile([128, D], F32, tag="mm")
                    for hh in range(2):
                        h = 2 * p2 + hh
                        o0 = 64 * hh
                        nc.tensor.matmul(
                            uv_ps[o0 : o0 + 64, :],
                            lhsT=hv_sb[o0 : o0 + 64, 64 * h : 64 * h + 64],
                            rhs=attn_sb[o0 : o0 + 64, :],
                            start=True,
                            stop=True,
                            tile_position=(o0, o0),
                        )
                    nc.scalar.copy(out=vecT_sb[:, p2, :], in_=uv_ps)

                # ---- out = vecT.T @ WoT --------------------------------
                out_sb = outp.tile([128, IT, D], F32, tag="o")
                for it in range(IT):
                    o_ps = mmp.tile([128, D], F32, tag="mm")
                    for hc in range(4):
                        nc.tensor.matmul(
                            o_ps,
                            lhsT=vecT_sb[:, hc, 128 * it : 128 * (it + 1)],
                            rhs=wo_sb[:, hc, :],
                            start=(hc == 0),
                            stop=(hc == 3),
                        )
                    nc.vector.tensor_copy(out=out_sb[:, it, :], in_=o_ps)
                nc.sync.dma_start(out=out_v[b, ic], in_=out_sb)

    return nc


_NC = None


def _get_nc() -> bass.Bass:
    global _NC
    if _NC is None:
        _NC = _build_nc()
    return _NC


def _consts() -> dict:
    poolD = np.zeros((128, 126), np.float32)
    poolD[0:64, 62] = 1.0 / 64.0
    poolD[64:128, 63] = 1.0 / 64.0
    ident = np.eye(128, dtype=np.float32)
    ones2 = np.zeros((128, 2), np.float32)
    ones2[0:64, 0] = 1.0
    ones2[64:128, 1] = 1.0
    expand2 = np.zeros((2, 128), np.float32)
    expand2[0, 0:64] = 1.0
    expand2[1, 64:128] = 1.0
    return dict(poolD=poolD, ident=ident, ones2=ones2, expand2=expand2)


def run(inputs: dict, trace: bool = False):
    """Run on 8 cores; returns (full_output, BassKernelResults)."""
    query = np.ascontiguousarray(np.asarray(inputs["query"], np.float32))
    keyvalue = np.ascontiguousarray(np.asarray(inputs["keyvalue"], np.float32))
    w = {
        "wqT": np.ascontiguousarray(np.asarray(inputs["Wq"], np.float32).T),
        "wkT": np.ascontiguousarray(np.asarray(inputs["Wk"], np.float32).T),
        "wvT": np.ascontiguousarray(np.asarray(inputs["Wv"], np.float32).T),
        "woT": np.ascontiguousarray(np.asarray(inputs["Wo"], np.float32).T),
    }
    consts = _consts()
    nb = query.shape[0]
    per = nb // NCORES
    assert per == B, f"expected {NCORES * B} batches, got {nb}"

    in_maps = []
    for k in range(NCORES):
        m = {
            "query": np.ascontiguousarray(query[k * per : (k + 1) * per]),
            "keyvalue": np.ascontiguousarray(keyvalue[k * per : (k + 1) * per]),
        }
        m.update(w)
        m.update(consts)
        in_maps.append(m)

    res = run_bass_kernel_spmd(
        _get_nc(), in_maps, core_ids=list(range(NCORES)), trace=trace
    )
    outs = [r["out"] for r in res.results]
    return np.concatenate(outs, axis=0), res


def kernel(**inputs) -> np.ndarray:
    out, _ = run(inputs, trace=False)
    return out


# revision 12
# speedup vs baseline: 2.3810x; 2.3810x over previous
"""Trainium2 Bass kernel for MultiHeadMemAttn (mean-pooled-memory attention).

Full computation (per batch b):
    mem  = mean_pool(keyvalue, window=64, stride=64)          # [64, 512]
    hq   = query @ Wq.T ; hk = mem @ Wk.T ; hv = mem @ Wv.T   # heads=8, hd=64
    attn = softmax(hq @ hk.T / 8, over mem axis)
    out  = (attn @ hv) @ Wo.T

Sharding: data-parallel over batch across 8 cores (4 batches each),
weights replicated.  No collectives.

Device strategy (per core):
  - all matmul operands use float32r (4-byte storage, reduced-precision PE
    path): 1 cycle/row instead of fp32's 4 cycles/row, ~7e-5 rel err.
  - pooling is a PE matmul against slices of a host-built band matrix,
    accumulated over 32 s-chunks in PSUM.
  - query tiles are PE-transposed to qT [d, i]; hqT = WqT-chunks.T @ qT.
  - scores computed transposed (scoresT[j, i] = hkT_h.T @ hqT_h), head
    pairs packed into one [128, 512] PSUM tile; softmax without
    max-subtraction (|scores| <= ~6): E = exp(s/8) on ACT, denominators via
    a K=128 ones-matmul -> [2,512], reciprocal_approx on DVE, partition
    broadcast on (idle) GPSIMD, normalize on DVE.
  - uvecT[dv, i] = V_h-as-lhsT.T @ attn (vecT comes out pre-transposed).
  - out[i, o] = vecT-chunks.T @ WoT in natural layout, DMA'd straight out.
"""

import os
from contextlib import ExitStack

import numpy as np

import concourse.bass as bass
import concourse.mybir as mybir
import concourse.tile as tile
from concourse.bass_utils import run_bass_kernel_spmd

F32 = mybir.dt.float32
F32R = mybir.dt.float32r

NCORES = 8
B = 4          # batches per core
QLEN = 1024
S = 4096       # kv sequence length
D = 512        # hidden
H = 8          # heads
HD = 64        # head dim
MEM = 64       # mem_len (pooled length)
DC = D // 128  # 4 chunks of the hidden dim
ICN = 2        # i-chunks of 512 per batch
IT = 4         # 128-row tiles per i-chunk
KT = 8         # kv DMA tiles per batch (4 s-chunks of 128 each)

EXPF = mybir.ActivationFunctionType.Exp

# ---------------------------------------------------------------------------
# Workaround: this walrus build only encodes ONE sem-wait per instruction
# ("Too many sync wait commands" in CoreV3GenImpl setupSyncWait), while
# Tile's sem-assignment freely attaches several.  Post-process the
# serialized BIR: move surplus waits onto injected same-engine NoOps placed
# immediately before the instruction (engine streams are in-order, so the
# NoOp chain stalls the engine exactly like multi-wait would).
import json as _json

_orig_to_json_bytes = bass.Bass.to_json_bytes


def _split_multi_waits(self, *args, **kwargs):
    raw = _orig_to_json_bytes(self, *args, **kwargs)
    d = _json.loads(raw)
    changed = False

    def fix_block(o):
        nonlocal changed
        if isinstance(o, dict):
            insts = o.get("instructions")
            if isinstance(insts, list):
                new = []
                for inst in insts:
                    si = inst.get("sync_info") if isinstance(inst, dict) else None
                    waits = (si or {}).get("on_wait") or []
                    if len(waits) > 1:
                        changed = True
                        for i, w in enumerate(waits[:-1]):
                            new.append(
                                {
                                    "name": f"{inst['name']}-sw{i}",
                                    "opcode": "NoOp",
                                    "engine": inst["engine"],
                                    "ins": [],
                                    "outs": [],
                                    "debug": inst.get("debug", 0),
                                    "sync_info": {
                                        "on_wait": [w],
                                        "on_update": [],
                                    },
                                }
                            )
                        si["on_wait"] = [waits[-1]]
                    new.append(inst)
                o["instructions"] = new
            for v in o.values():
                fix_block(v)
        elif isinstance(o, list):
            for v in o:
                fix_block(v)

    fix_block(d)
    if not changed:
        return raw
    return _json.dumps(d).encode()


bass.Bass.to_json_bytes = _split_multi_waits
# ---------------------------------------------------------------------------


def _build_nc() -> bass.Bass:
    nc = bass.Bass()
    # inputs declared float32r: same 4-byte layout as the f32 numpy arrays
    # we feed in; the PE reads them at reduced precision / full speed.
    q = nc.dram_tensor("query", [B, QLEN, D], F32R, kind="ExternalInput")
    kv = nc.dram_tensor("keyvalue", [B, S, D], F32R, kind="ExternalInput")
    wqT = nc.dram_tensor("wqT", [D, D], F32R, kind="ExternalInput")
    wkT = nc.dram_tensor("wkT", [D, D], F32R, kind="ExternalInput")
    wvT = nc.dram_tensor("wvT", [D, D], F32R, kind="ExternalInput")
    woT = nc.dram_tensor("woT", [D, D], F32R, kind="ExternalInput")
    poolD = nc.dram_tensor("poolD", [128, 126], F32R, kind="ExternalInput")
    ident = nc.dram_tensor("ident", [128, 128], F32R, kind="ExternalInput")
    ones2 = nc.dram_tensor("ones2", [128, 4, 32], F32R, kind="ExternalInput")
    expand2 = nc.dram_tensor("expand2", [32, 4, 128], F32R, kind="ExternalInput")
    out = nc.dram_tensor("out", [B, QLEN, D], F32, kind="ExternalOutput")

    # DRAM views for partition-major DMA
    q_v = q.rearrange("b (ic it p) d -> b ic p it d", ic=ICN, it=IT, p=128)
    kv_v = kv.rearrange("b (t c p) d -> b t p c d", t=KT, c=4, p=128)
    out_v = out.rearrange("b (ic it p) d -> b ic p it d", ic=ICN, it=IT, p=128)
    wq_v = wqT.rearrange("(dc p) o -> p dc o", p=128)
    wk_v = wkT.rearrange("(dc p) o -> p dc o", p=128)
    wv_v = wvT.rearrange("(dc p) o -> p dc o", p=128)
    wo_v = woT.rearrange("(dc p) o -> p dc o", p=128)

    with tile.TileContext(nc) as tc, ExitStack() as ctx:
        # SBUF pools
        singles = ctx.enter_context(tc.tile_pool(name="singles", bufs=1))
        kvp = ctx.enter_context(tc.tile_pool(name="kvp", bufs=2))
        qp = ctx.enter_context(tc.tile_pool(name="qp", bufs=2))
        qtp = ctx.enter_context(tc.tile_pool(name="qtp", bufs=2))
        hqp = ctx.enter_context(tc.tile_pool(name="hqp", bufs=2))
        memp = ctx.enter_context(tc.tile_pool(name="memp", bufs=2))
        ep = ctx.enter_context(tc.tile_pool(name="ep", bufs=6))
        attnp = ctx.enter_context(tc.tile_pool(name="attnp", bufs=3))
        rdp = ctx.enter_context(tc.tile_pool(name="rdp", bufs=3))
        bcp = ctx.enter_context(tc.tile_pool(name="bcp", bufs=3))
        vtp = ctx.enter_context(tc.tile_pool(name="vtp", bufs=2))
        outp = ctx.enter_context(tc.tile_pool(name="outp", bufs=2))
        # PSUM pools (8 banks total: 1 + 2 + 1 + 4)
        accp = ctx.enter_context(tc.tile_pool(name="accp", bufs=1, space="PSUM"))
        trp = ctx.enter_context(tc.tile_pool(name="trp", bufs=2, space="PSUM"))
        denp = ctx.enter_context(tc.tile_pool(name="denp", bufs=1, space="PSUM"))
        mmp = ctx.enter_context(tc.tile_pool(name="mmp", bufs=4, space="PSUM"))

        # one-time loads
        wq_sb = singles.tile([128, DC, D], F32R)
        nc.sync.dma_start(out=wq_sb, in_=wq_v)
        wk_sb = singles.tile([128, DC, D], F32R)
        nc.sync.dma_start(out=wk_sb, in_=wk_v)
        wv_sb = singles.tile([128, DC, D], F32R)
        nc.sync.dma_start(out=wv_sb, in_=wv_v)
        wo_sb = singles.tile([128, DC, D], F32R)
        nc.sync.dma_start(out=wo_sb, in_=wo_v)
        poolD_sb = singles.tile([128, 126], F32R)
        nc.sync.dma_start(out=poolD_sb, in_=poolD[:, :])
        ident_sb = singles.tile([128, 128], F32R)
        nc.sync.dma_start(out=ident_sb, in_=ident[:, :])
        ones2_sb = singles.tile([128, 4, 32], F32R)
        nc.sync.dma_start(out=ones2_sb, in_=ones2[:, :, :])
        expand2_sb = singles.tile([32, 4, 128], F32R)
        nc.sync.dma_start(out=expand2_sb, in_=expand2[:, :, :])

        for b in range(B):
            # ---- mean-pool keyvalue -> mem [64, 512] -------------------
            pacc = accp.tile([MEM, D], F32, tag="acc")
            for t in range(KT):
                kvt = kvp.tile([128, 4, D], F32R, tag="kv")
                nc.sync.dma_start(out=kvt, in_=kv_v[b, t])
                for c in range(4):
                    sc = 4 * t + c
                    nc.tensor.matmul(
                        pacc,
                        lhsT=poolD_sb[:, 62 - 2 * sc : 126 - 2 * sc],
                        rhs=kvt[:, c, :],
                        start=(sc == 0),
                        stop=(sc == 31),
                    )
            mem_sb = memp.tile([MEM, D], F32R, tag="mem")
            nc.scalar.copy(out=mem_sb, in_=pacc)

            # ---- memT [d, m] (4 chunks of 128 d) -----------------------
            trt = trp.tile([128, 4, MEM], F32R, tag="tr")
            for c in range(4):
                nc.tensor.transpose(
                    trt[:, c, :],
                    mem_sb[:, 128 * c : 128 * (c + 1)],
                    ident_sb[0:MEM, 0:MEM],
                )
            memT_sb = memp.tile([128, 4, MEM], F32R, tag="memT")
            nc.scalar.copy(out=memT_sb, in_=trt)

            # ---- hkT, stored block-diagonal per head pair --------------
            # hkbd[:, p2, :] is [128, 128]: [0:64, 0:64] = hkT of even head
            # (scaled by 1/8), [64:128, 64:128] = odd head, zero elsewhere.
            hkbd_sb = memp.tile([128, 4, 128], F32R, tag="hkbd")
            nc.scalar.mul(out=hkbd_sb, in_=wk_sb[:, 0, :], mul=0.0)
            for oc in range(4):
                hk_ps = mmp.tile([128, MEM], F32, tag="mm")
                for dc in range(DC):
                    nc.tensor.matmul(
                        hk_ps,
                        lhsT=wk_sb[:, dc, 128 * oc : 128 * (oc + 1)],
                        rhs=memT_sb[:, dc, :],
                        start=(dc == 0),
                        stop=(dc == DC - 1),
                    )
                nc.scalar.mul(
                    out=hkbd_sb[0:64, oc, 0:64], in_=hk_ps[0:64, :], mul=0.125
                )
                nc.scalar.mul(
                    out=hkbd_sb[64:128, oc, 64:128], in_=hk_ps[64:128, :], mul=0.125
                )

            # ---- hv [m, o]; then block-diagonal per head pair ----------
            # hvbd[:, p2, :]: [0:64, 0:64] = V_even [j, dv], [64:128, 64:128]
            # = V_odd, zero elsewhere (odd blocks moved across partitions
            # with a small SBUF->SBUF DMA).
            hv_ps = mmp.tile([MEM, D], F32, tag="mm")
            for dc in range(DC):
                nc.tensor.matmul(
                    hv_ps,
                    lhsT=memT_sb[:, dc, :],
                    rhs=wv_sb[:, dc, :],
                    start=(dc == 0),
                    stop=(dc == DC - 1),
                )
            hv_sb = memp.tile([MEM, D], F32R, tag="hv")
            nc.scalar.copy(out=hv_sb, in_=hv_ps)
            hvbd_sb = memp.tile([128, 4, 128], F32R, tag="hvbd")
            nc.scalar.mul(out=hvbd_sb, in_=wv_sb[:, 0, :], mul=0.0)
            ev = hv_sb.rearrange("m (p2 two dv) -> m p2 two dv", p2=4, two=2)
            nc.scalar.copy(out=hvbd_sb[0:64, :, 0:64], in_=ev[:, :, 0, :])
            nc.sync.dma_start(out=hvbd_sb[64:128, :, 64:128], in_=ev[:, :, 1, :])

            for ic in range(ICN):
                # ---- load q, transpose to qT [d, i] --------------------
                qt = qp.tile([128, IT, D], F32R, tag="q")
                nc.sync.dma_start(out=qt, in_=q_v[b, ic])
                qT_sb = qtp.tile([128, DC, D], F32R, tag="qT")
                for it in range(IT):
                    trq = trp.tile([128, 4, 128], F32R, tag="tr")
                    for c in range(4):
                        nc.tensor.transpose(
                            trq[:, c, :],
                            qt[:, it, 128 * c : 128 * (c + 1)],
                            ident_sb,
                        )
                    nc.scalar.copy(
                        out=qT_sb[:, :, 128 * it : 128 * (it + 1)], in_=trq
                    )

                # ---- hqT [o, i] ----------------------------------------
                hqT_sb = hqp.tile([128, DC, D], F32R, tag="hqT")
                for oc in range(DC):
                    hq_ps = mmp.tile([128, D], F32, tag="mm")
                    for dc in range(DC):
                        nc.tensor.matmul(
                            hq_ps,
                            lhsT=wq_sb[:, dc, 128 * oc : 128 * (oc + 1)],
                            rhs=qT_sb[:, dc, :],
                            start=(dc == 0),
                            stop=(dc == DC - 1),
                        )
                    nc.scalar.copy(out=hqT_sb[:, oc, :], in_=hq_ps)

                # ---- attention: scores/exp for all 4 head pairs (one
                # block-diagonal K=128 matmul each), denominators
                # accumulated into one [32, D] tile, one batched
                # reciprocal, then normalize + blockdiag V matmul
                vecT_sb = vtp.tile([128, 4, D], F32R, tag="vecT")
                den_ps = denp.tile([32, D], F32, tag="den")
                e_tiles = []
                for p2 in range(4):
                    sc_ps = mmp.tile([128, D], F32, tag="mm")
                    nc.tensor.matmul(
                        sc_ps,
                        lhsT=hkbd_sb[:, p2, :],
                        rhs=hqT_sb[:, p2, :],
                        start=True,
                        stop=True,
                    )
                    e_sb = ep.tile([128, D], F32R, tag="e")
                    nc.scalar.activation(out=e_sb, in_=sc_ps, func=EXPF)
                    e_tiles.append(e_sb)
                    nc.tensor.matmul(
                        den_ps,
                        lhsT=ones2_sb[:, p2, :],
                        rhs=e_sb,
                        start=(p2 == 0),
                        stop=(p2 == 3),
                    )
                rden_r = rdp.tile([32, D], F32R, tag="rden")
                with nc.allow_low_precision(reason="f32r reciprocal feeds f32r matmul"):
                    nc.vector.reciprocal(out=rden_r, in_=den_ps)
                for p2 in range(4):
                    bc_ps = mmp.tile([128, D], F32, tag="mm")
                    nc.tensor.matmul(
                        bc_ps,
                        lhsT=expand2_sb[:, p2, :],
                        rhs=rden_r,
                        start=True,
                        stop=True,
                    )
                    attn_sb = attnp.tile([128, D], F32R, tag="attn")
                    nc.vector.tensor_mul(attn_sb, e_tiles[p2], bc_ps)
                    uv_ps = mmp.tile([128, D], F32, tag="mm")
                    nc.tensor.matmul(
                        uv_ps,
                        lhsT=hvbd_sb[:, p2, :],
                        rhs=attn_sb,
                        start=True,
                        stop=True,
                    )
                    nc.scalar.copy(out=vecT_sb[:, p2, :], in_=uv_ps)

                # ---- out = vecT.T @ WoT --------------------------------
                out_sb = outp.tile([128, IT, D], F32, tag="o")
                for it in range(IT):
                    o_ps = mmp.tile([128, D], F32, tag="mm")
                    for hc in range(4):
                        nc.tensor.matmul(
                            o_ps,
                            lhsT=vecT_sb[:, hc, 128 * it : 128 * (it + 1)],
                            rhs=wo_sb[:, hc, :],
                            start=(hc == 0),
                            stop=(hc == 3),
                        )
                    nc.vector.tensor_copy(out=out_sb[:, it, :], in_=o_ps)
                # store on the second HWDGE ring (ACT) so loads/stores overlap
                nc.scalar.dma_start(out=out_v[b, ic], in_=out_sb)

    return nc


_NC = None


def _get_nc() -> bass.Bass:
    global _NC
    if _NC is None:
        _NC = _build_nc()
    return _NC


def _consts() -> dict:
    poolD = np.zeros((128, 126), np.float32)
    poolD[0:64, 62] = 1.0 / 64.0
    poolD[64:128, 63] = 1.0 / 64.0
    ident = np.eye(128, dtype=np.float32)
    # den matmul lhsT (per pair p2): accumulate into one [32, D] tile; row
    # 2*p2 sums even-head exp rows (partitions 0-63), row 2*p2+1 sums odd
    # (64-127); rows 8-31 duplicate row 0's pattern so every PSUM row gets
    # a finite positive value (reciprocal runs on the whole tile).
    ones2 = np.zeros((128, 4, 32), np.float32)
    for p2 in range(4):
        ones2[0:64, p2, 2 * p2] = 1.0
        ones2[64:128, p2, 2 * p2 + 1] = 1.0
        if p2 == 0:
            ones2[0:64, p2, 8:32] = 1.0
    # broadcast matmul lhsT, one [32, 128] slice per head pair
    expand2 = np.zeros((32, 4, 128), np.float32)
    for p2 in range(4):
        expand2[2 * p2 + 0, p2, 0:64] = 1.0
        expand2[2 * p2 + 1, p2, 64:128] = 1.0
    return dict(poolD=poolD, ident=ident, ones2=ones2, expand2=expand2)


def run(inputs: dict, trace: bool = False):
    """Run on 8 cores; returns (full_output, BassKernelResults)."""
    query = np.ascontiguousarray(np.asarray(inputs["query"], np.float32))
    keyvalue = np.ascontiguousarray(np.asarray(inputs["keyvalue"], np.float32))
    w = {
        "wqT": np.ascontiguousarray(np.asarray(inputs["Wq"], np.float32).T),
        "wkT": np.ascontiguousarray(np.asarray(inputs["Wk"], np.float32).T),
        "wvT": np.ascontiguousarray(np.asarray(inputs["Wv"], np.float32).T),
        "woT": np.ascontiguousarray(np.asarray(inputs["Wo"], np.float32).T),
    }
    consts = _consts()
    nb = query.shape[0]
    per = nb // NCORES
    assert per == B, f"expected {NCORES * B} batches, got {nb}"

    in_maps = []
    for k in range(NCORES):
        m = {
            "query": np.ascontiguousarray(query[k * per : (k + 1) * per]),
            "keyvalue": np.ascontiguousarray(keyvalue[k * per : (k + 1) * per]),
        }
        m.update(w)
        m.update(consts)
        in_maps.append(m)

    res = run_bass_kernel_spmd(
        _get_nc(), in_maps, core_ids=list(range(NCORES)), trace=trace
    )
    outs = [r["out"] for r in res.results]
    return np.concatenate(outs, axis=0), res


def kernel(**inputs) -> np.ndarray:
    out, _ = run(inputs, trace=False)
    return out


# revision 14
# speedup vs baseline: 2.4013x; 1.0085x over previous
"""Trainium2 Bass kernel for MultiHeadMemAttn (mean-pooled-memory attention).

Full computation (per batch b):
    mem  = mean_pool(keyvalue, window=64, stride=64)          # [64, 512]
    hq   = query @ Wq.T ; hk = mem @ Wk.T ; hv = mem @ Wv.T   # heads=8, hd=64
    attn = softmax(hq @ hk.T / 8, over mem axis)
    out  = (attn @ hv) @ Wo.T

Sharding: data-parallel over batch across 8 cores (4 batches each),
weights replicated.  No collectives.

Device strategy (per core):
  - all matmul operands use float32r (4-byte storage, reduced-precision PE
    path): 1 cycle/row instead of fp32's 4 cycles/row, ~7e-5 rel err.
  - pooling is a PE matmul against slices of a host-built band matrix,
    accumulated over 32 s-chunks in PSUM.
  - query tiles are PE-transposed to qT [d, i]; hqT = WqT-chunks.T @ qT.
  - scores computed transposed (scoresT[j, i] = hkT_h.T @ hqT_h), head
    pairs packed into one [128, 512] PSUM tile; softmax without
    max-subtraction (|scores| <= ~6): E = exp(s/8) on ACT, denominators via
    a K=128 ones-matmul -> [2,512], reciprocal_approx on DVE, partition
    broadcast on (idle) GPSIMD, normalize on DVE.
  - uvecT[dv, i] = V_h-as-lhsT.T @ attn (vecT comes out pre-transposed).
  - out[i, o] = vecT-chunks.T @ WoT in natural layout, DMA'd straight out.
"""

import os
from contextlib import ExitStack

import numpy as np

import concourse.bass as bass
import concourse.mybir as mybir
import concourse.tile as tile
from concourse.bass_utils import run_bass_kernel_spmd

F32 = mybir.dt.float32
F32R = mybir.dt.float32r

NCORES = 8
B = 4          # batches per core
QLEN = 1024
S = 4096       # kv sequence length
D = 512        # hidden
H = 8          # heads
HD = 64        # head dim
MEM = 64       # mem_len (pooled length)
DC = D // 128  # 4 chunks of the hidden dim
ICN = 2        # i-chunks of 512 per batch
IT = 4         # 128-row tiles per i-chunk
KT = 4         # kv DMA tiles per batch (8 s-chunks of 128 each)

EXPF = mybir.ActivationFunctionType.Exp

# ---------------------------------------------------------------------------
# Workaround: this walrus build only encodes ONE sem-wait per instruction
# ("Too many sync wait commands" in CoreV3GenImpl setupSyncWait), while
# Tile's sem-assignment freely attaches several.  Post-process the
# serialized BIR: move surplus waits onto injected same-engine NoOps placed
# immediately before the instruction (engine streams are in-order, so the
# NoOp chain stalls the engine exactly like multi-wait would).
import json as _json

_orig_to_json_bytes = bass.Bass.to_json_bytes


def _split_multi_waits(self, *args, **kwargs):
    raw = _orig_to_json_bytes(self, *args, **kwargs)
    d = _json.loads(raw)
    changed = False

    def fix_block(o):
        nonlocal changed
        if isinstance(o, dict):
            insts = o.get("instructions")
            if isinstance(insts, list):
                new = []
                for inst in insts:
                    si = inst.get("sync_info") if isinstance(inst, dict) else None
                    waits = (si or {}).get("on_wait") or []
                    if len(waits) > 1:
                        changed = True
                        for i, w in enumerate(waits[:-1]):
                            new.append(
                                {
                                    "name": f"{inst['name']}-sw{i}",
                                    "opcode": "NoOp",
                                    "engine": inst["engine"],
                                    "ins": [],
                                    "outs": [],
                                    "debug": inst.get("debug", 0),
                                    "sync_info": {
                                        "on_wait": [w],
                                        "on_update": [],
                                    },
                                }
                            )
                        si["on_wait"] = [waits[-1]]
                    new.append(inst)
                o["instructions"] = new
            for v in o.values():
                fix_block(v)
        elif isinstance(o, list):
            for v in o:
                fix_block(v)

    fix_block(d)
    if not changed:
        return raw
    return _json.dumps(d).encode()


bass.Bass.to_json_bytes = _split_multi_waits
# ---------------------------------------------------------------------------


def _build_nc() -> bass.Bass:
    nc = bass.Bass()
    # inputs declared float32r: same 4-byte layout as the f32 numpy arrays
    # we feed in; the PE reads them at reduced precision / full speed.
    q = nc.dram_tensor("queryT", [B, D, QLEN], F32R, kind="ExternalInput")
    kv = nc.dram_tensor("keyvalue", [B, S, D], F32R, kind="ExternalInput")
    wqT = nc.dram_tensor("wqT", [D, D], F32R, kind="ExternalInput")
    wkT = nc.dram_tensor("wkT", [D, D], F32R, kind="ExternalInput")
    wvT = nc.dram_tensor("wvT", [D, D], F32R, kind="ExternalInput")
    woT = nc.dram_tensor("woT", [D, D], F32R, kind="ExternalInput")
    poolD = nc.dram_tensor("poolD", [128, 126], F32R, kind="ExternalInput")
    ident = nc.dram_tensor("ident", [128, 128], F32R, kind="ExternalInput")
    ones2 = nc.dram_tensor("ones2", [128, 4, 32], F32R, kind="ExternalInput")
    expand2 = nc.dram_tensor("expand2", [32, 4, 128], F32R, kind="ExternalInput")
    out = nc.dram_tensor("out", [B, QLEN, D], F32, kind="ExternalOutput")

    # DRAM views for partition-major DMA
    q_v = q.rearrange("b (dc p) (ic i) -> b ic p dc i", p=128, ic=ICN)
    kv_v = kv.rearrange("b (t c p) d -> b t p c d", t=KT, c=8, p=128)
    out_v = out.rearrange("b (ic it p) d -> b ic p it d", ic=ICN, it=IT, p=128)
    wq_v = wqT.rearrange("(dc p) o -> p dc o", p=128)
    wk_v = wkT.rearrange("(dc p) o -> p dc o", p=128)
    wv_v = wvT.rearrange("(dc p) o -> p dc o", p=128)
    wo_v = woT.rearrange("(dc p) o -> p dc o", p=128)

    with tile.TileContext(nc) as tc, ExitStack() as ctx:
        # SBUF pools
        singles = ctx.enter_context(tc.tile_pool(name="singles", bufs=1))
        kvp = ctx.enter_context(tc.tile_pool(name="kvp", bufs=2))
        qtp = ctx.enter_context(tc.tile_pool(name="qtp", bufs=2))
        hqp = ctx.enter_context(tc.tile_pool(name="hqp", bufs=2))
        memp = ctx.enter_context(tc.tile_pool(name="memp", bufs=2))
        ep = ctx.enter_context(tc.tile_pool(name="ep", bufs=6))
        attnp = ctx.enter_context(tc.tile_pool(name="attnp", bufs=3))
        rdp = ctx.enter_context(tc.tile_pool(name="rdp", bufs=3))
        bcp = ctx.enter_context(tc.tile_pool(name="bcp", bufs=3))
        vtp = ctx.enter_context(tc.tile_pool(name="vtp", bufs=2))
        outp = ctx.enter_context(tc.tile_pool(name="outp", bufs=2))
        # PSUM pools (8 banks total: 1 + 1 + 1 + 5)
        accp = ctx.enter_context(tc.tile_pool(name="accp", bufs=1, space="PSUM"))
        trp = ctx.enter_context(tc.tile_pool(name="trp", bufs=1, space="PSUM"))
        denp = ctx.enter_context(tc.tile_pool(name="denp", bufs=1, space="PSUM"))
        mmp = ctx.enter_context(tc.tile_pool(name="mmp", bufs=5, space="PSUM"))

        # one-time loads
        wq_sb = singles.tile([128, DC, D], F32R)
        nc.sync.dma_start(out=wq_sb, in_=wq_v)
        wk_sb = singles.tile([128, DC, D], F32R)
        nc.sync.dma_start(out=wk_sb, in_=wk_v)
        wv_sb = singles.tile([128, DC, D], F32R)
        nc.sync.dma_start(out=wv_sb, in_=wv_v)
        wo_sb = singles.tile([128, DC, D], F32R)
        nc.sync.dma_start(out=wo_sb, in_=wo_v)
        poolD_sb = singles.tile([128, 126], F32R)
        nc.sync.dma_start(out=poolD_sb, in_=poolD[:, :])
        ident_sb = singles.tile([128, 128], F32R)
        nc.sync.dma_start(out=ident_sb, in_=ident[:, :])
        ones2_sb = singles.tile([128, 4, 32], F32R)
        nc.sync.dma_start(out=ones2_sb, in_=ones2[:, :, :])
        expand2_sb = singles.tile([32, 4, 128], F32R)
        nc.sync.dma_start(out=expand2_sb, in_=expand2[:, :, :])

        for b in range(B):
            # ---- mean-pool keyvalue -> mem [64, 512] -------------------
            pacc = accp.tile([MEM, D], F32, tag="acc")
            for t in range(KT):
                kvt = kvp.tile([128, 8, D], F32R, tag="kv")
                nc.sync.dma_start(out=kvt, in_=kv_v[b, t])
                for c in range(8):
                    sc = 8 * t + c
                    nc.tensor.matmul(
                        pacc,
                        lhsT=poolD_sb[:, 62 - 2 * sc : 126 - 2 * sc],
                        rhs=kvt[:, c, :],
                        start=(sc == 0),
                        stop=(sc == 31),
                    )
            mem_sb = memp.tile([MEM, D], F32R, tag="mem")
            nc.scalar.copy(out=mem_sb, in_=pacc)

            # ---- memT [d, m] (4 chunks of 128 d) -----------------------
            trt = trp.tile([128, 4, MEM], F32R, tag="tr")
            for c in range(4):
                nc.tensor.transpose(
                    trt[:, c, :],
                    mem_sb[:, 128 * c : 128 * (c + 1)],
                    ident_sb[0:MEM, 0:MEM],
                )
            memT_sb = memp.tile([128, 4, MEM], F32R, tag="memT")
            nc.scalar.copy(out=memT_sb, in_=trt)

            # ---- hkT, stored block-diagonal per head pair --------------
            # hkbd[:, p2, :] is [128, 128]: [0:64, 0:64] = hkT of even head
            # (scaled by 1/8), [64:128, 64:128] = odd head, zero elsewhere.
            hkbd_sb = memp.tile([128, 4, 128], F32R, tag="hkbd")
            nc.scalar.mul(out=hkbd_sb, in_=wk_sb[:, 0, :], mul=0.0)
            for oc in range(4):
                hk_ps = mmp.tile([128, MEM], F32, tag="mm")
                for dc in range(DC):
                    nc.tensor.matmul(
                        hk_ps,
                        lhsT=wk_sb[:, dc, 128 * oc : 128 * (oc + 1)],
                        rhs=memT_sb[:, dc, :],
                        start=(dc == 0),
                        stop=(dc == DC - 1),
                    )
                nc.scalar.mul(
                    out=hkbd_sb[0:64, oc, 0:64], in_=hk_ps[0:64, :], mul=0.125
                )
                nc.scalar.mul(
                    out=hkbd_sb[64:128, oc, 64:128], in_=hk_ps[64:128, :], mul=0.125
                )

            # ---- hv [m, o]; then block-diagonal per head pair ----------
            # hvbd[:, p2, :]: [0:64, 0:64] = V_even [j, dv], [64:128, 64:128]
            # = V_odd, zero elsewhere (odd blocks moved across partitions
            # with a small SBUF->SBUF DMA).
            hv_ps = mmp.tile([MEM, D], F32, tag="mm")
            for dc in range(DC):
                nc.tensor.matmul(
                    hv_ps,
                    lhsT=memT_sb[:, dc, :],
                    rhs=wv_sb[:, dc, :],
                    start=(dc == 0),
                    stop=(dc == DC - 1),
                )
            hv_sb = memp.tile([MEM, D], F32R, tag="hv")
            nc.scalar.copy(out=hv_sb, in_=hv_ps)
            hvbd_sb = memp.tile([128, 4, 128], F32R, tag="hvbd")
            nc.scalar.mul(out=hvbd_sb, in_=wv_sb[:, 0, :], mul=0.0)
            ev = hv_sb.rearrange("m (p2 two dv) -> m p2 two dv", p2=4, two=2)
            nc.scalar.copy(out=hvbd_sb[0:64, :, 0:64], in_=ev[:, :, 0, :])
            nc.sync.dma_start(out=hvbd_sb[64:128, :, 64:128], in_=ev[:, :, 1, :])

            for ic in range(ICN):
                # ---- load qT [d, i] (transposed host-side) -------------
                qT_sb = qtp.tile([128, DC, D], F32R, tag="qT")
                nc.sync.dma_start(out=qT_sb, in_=q_v[b, ic])

                # ---- hqT [o, i] ----------------------------------------
                hqT_sb = hqp.tile([128, DC, D], F32R, tag="hqT")
                for oc in range(DC):
                    hq_ps = mmp.tile([128, D], F32, tag="mm")
                    for dc in range(DC):
                        nc.tensor.matmul(
                            hq_ps,
                            lhsT=wq_sb[:, dc, 128 * oc : 128 * (oc + 1)],
                            rhs=qT_sb[:, dc, :],
                            start=(dc == 0),
                            stop=(dc == DC - 1),
                        )
                    nc.scalar.copy(out=hqT_sb[:, oc, :], in_=hq_ps)

                # ---- attention: scores/exp for all 4 head pairs (one
                # block-diagonal K=128 matmul each), denominators
                # accumulated into one [32, D] tile, one batched
                # reciprocal, then normalize + blockdiag V matmul
                vecT_sb = vtp.tile([128, 4, D], F32R, tag="vecT")
                den_ps = denp.tile([32, D], F32, tag="den")
                e_tiles = []
                for p2 in range(4):
                    sc_ps = mmp.tile([128, D], F32, tag="mm")
                    nc.tensor.matmul(
                        sc_ps,
                        lhsT=hkbd_sb[:, p2, :],
                        rhs=hqT_sb[:, p2, :],
                        start=True,
                        stop=True,
                    )
                    e_sb = ep.tile([128, D], F32R, tag="e")
                    nc.scalar.activation(out=e_sb, in_=sc_ps, func=EXPF)
                    e_tiles.append(e_sb)
                    nc.tensor.matmul(
                        den_ps,
                        lhsT=ones2_sb[:, p2, :],
                        rhs=e_sb,
                        start=(p2 == 0),
                        stop=(p2 == 3),
                    )
                rden_r = rdp.tile([32, D], F32R, tag="rden")
                with nc.allow_low_precision(reason="f32r reciprocal feeds f32r matmul"):
                    nc.vector.reciprocal(out=rden_r, in_=den_ps)
                for p2 in range(4):
                    bc_ps = mmp.tile([128, D], F32, tag="mm")
                    nc.tensor.matmul(
                        bc_ps,
                        lhsT=expand2_sb[:, p2, :],
                        rhs=rden_r,
                        start=True,
                        stop=True,
                    )
                    attn_sb = attnp.tile([128, D], F32R, tag="attn")
                    nc.vector.tensor_mul(attn_sb, e_tiles[p2], bc_ps)
                    uv_ps = mmp.tile([128, D], F32, tag="mm")
                    nc.tensor.matmul(
                        uv_ps,
                        lhsT=hvbd_sb[:, p2, :],
                        rhs=attn_sb,
                        start=True,
                        stop=True,
                    )
                    nc.scalar.copy(out=vecT_sb[:, p2, :], in_=uv_ps)

                # ---- out = vecT.T @ WoT --------------------------------
                out_sb = outp.tile([128, IT, D], F32, tag="o")
                for it in range(IT):
                    o_ps = mmp.tile([128, D], F32, tag="mm")
                    for hc in range(4):
                        nc.tensor.matmul(
                            o_ps,
                            lhsT=vecT_sb[:, hc, 128 * it : 128 * (it + 1)],
                            rhs=wo_sb[:, hc, :],
                            start=(hc == 0),
                            stop=(hc == 3),
                        )
                    nc.vector.tensor_copy(out=out_sb[:, it, :], in_=o_ps)
                # store on the second HWDGE ring (ACT) so loads/stores overlap
                nc.scalar.dma_start(out=out_v[b, ic], in_=out_sb)

    return nc


_NC = None


def _get_nc() -> bass.Bass:
    global _NC
    if _NC is None:
        _NC = _build_nc()
    return _NC


def _consts() -> dict:
    poolD = np.zeros((128, 126), np.float32)
    poolD[0:64, 62] = 1.0 / 64.0
    poolD[64:128, 63] = 1.0 / 64.0
    ident = np.eye(128, dtype=np.float32)
    # den matmul lhsT (per pair p2): accumulate into one [32, D] tile; row
    # 2*p2 sums even-head exp rows (partitions 0-63), row 2*p2+1 sums odd
    # (64-127); rows 8-31 duplicate row 0's pattern so every PSUM row gets
    # a finite positive value (reciprocal runs on the whole tile).
    ones2 = np.zeros((128, 4, 32), np.float32)
    for p2 in range(4):
        ones2[0:64, p2, 2 * p2] = 1.0
        ones2[64:128, p2, 2 * p2 + 1] = 1.0
        if p2 == 0:
            ones2[0:64, p2, 8:32] = 1.0
    # broadcast matmul lhsT, one [32, 128] slice per head pair
    expand2 = np.zeros((32, 4, 128), np.float32)
    for p2 in range(4):
        expand2[2 * p2 + 0, p2, 0:64] = 1.0
        expand2[2 * p2 + 1, p2, 64:128] = 1.0
    return dict(poolD=poolD, ident=ident, ones2=ones2, expand2=expand2)


def run(inputs: dict, trace: bool = False):
    """Run on 8 cores; returns (full_output, BassKernelResults)."""
    query = np.asarray(inputs["query"], np.float32)
    queryT = np.ascontiguousarray(query.transpose(0, 2, 1))
    keyvalue = np.ascontiguousarray(np.asarray(inputs["keyvalue"], np.float32))
    w = {
        "wqT": np.ascontiguousarray(np.asarray(inputs["Wq"], np.float32).T),
        "wkT": np.ascontiguousarray(np.asarray(inputs["Wk"], np.float32).T),
        "wvT": np.ascontiguousarray(np.asarray(inputs["Wv"], np.float32).T),
        "woT": np.ascontiguousarray(np.asarray(inputs["Wo"], np.float32).T),
    }
    consts = _consts()
    nb = query.shape[0]
    per = nb // NCORES
    assert per == B, f"expected {NCORES * B} batches, got {nb}"

    in_maps = []
    for k in range(NCORES):
        m = {
            "queryT": np.ascontiguousarray(queryT[k * per : (k + 1) * per]),
            "keyvalue": np.ascontiguousarray(keyvalue[k * per : (k + 1) * per]),
        }
        m.update(w)
        m.update(consts)
        in_maps.append(m)

    res = run_bass_kernel_spmd(
        _get_nc(), in_maps, core_ids=list(range(NCORES)), trace=trace
    )
    outs = [r["out"] for r in res.results]
    return np.concatenate(outs, axis=0), res


def kernel(**inputs) -> np.ndarray:
    out, _ = run(inputs, trace=False)
    return out


# revision 15
# speedup vs baseline: 2.4748x; 1.0306x over previous
"""Trainium2 Bass kernel for MultiHeadMemAttn (mean-pooled-memory attention).

Full computation (per batch b):
    mem  = mean_pool(keyvalue, window=64, stride=64)          # [64, 512]
    hq   = query @ Wq.T ; hk = mem @ Wk.T ; hv = mem @ Wv.T   # heads=8, hd=64
    attn = softmax(hq @ hk.T / 8, over mem axis)
    out  = (attn @ hv) @ Wo.T

Sharding: data-parallel over batch across 8 cores (4 batches each),
weights replicated.  No collectives.

Device strategy (per core):
  - all matmul operands use float32r (4-byte storage, reduced-precision PE
    path): 1 cycle/row instead of fp32's 4 cycles/row, ~7e-5 rel err.
  - pooling is a PE matmul against slices of a host-built band matrix,
    accumulated over 32 s-chunks in PSUM.
  - query tiles are PE-transposed to qT [d, i]; hqT = WqT-chunks.T @ qT.
  - scores computed transposed (scoresT[j, i] = hkT_h.T @ hqT_h), head
    pairs packed into one [128, 512] PSUM tile; softmax without
    max-subtraction (|scores| <= ~6): E = exp(s/8) on ACT, denominators via
    a K=128 ones-matmul -> [2,512], reciprocal_approx on DVE, partition
    broadcast on (idle) GPSIMD, normalize on DVE.
  - uvecT[dv, i] = V_h-as-lhsT.T @ attn (vecT comes out pre-transposed).
  - out[i, o] = vecT-chunks.T @ WoT in natural layout, DMA'd straight out.
"""

import os
from contextlib import ExitStack

import numpy as np

import concourse.bass as bass
import concourse.mybir as mybir
import concourse.tile as tile
from concourse.bass_utils import run_bass_kernel_spmd

F32 = mybir.dt.float32
F32R = mybir.dt.float32r

NCORES = 8
B = 4          # batches per core
QLEN = 1024
S = 4096       # kv sequence length
D = 512        # hidden
H = 8          # heads
HD = 64        # head dim
MEM = 64       # mem_len (pooled length)
DC = D // 128  # 4 chunks of the hidden dim
ICN = 2        # i-chunks of 512 per batch
IT = 4         # 128-row tiles per i-chunk
KT = 4         # kv DMA tiles per batch (8 s-chunks of 128 each)

EXPF = mybir.ActivationFunctionType.Exp

# ---------------------------------------------------------------------------
# Workaround: this walrus build only encodes ONE sem-wait per instruction
# ("Too many sync wait commands" in CoreV3GenImpl setupSyncWait), while
# Tile's sem-assignment freely attaches several.  Post-process the
# serialized BIR: move surplus waits onto injected same-engine NoOps placed
# immediately before the instruction (engine streams are in-order, so the
# NoOp chain stalls the engine exactly like multi-wait would).
import json as _json

_orig_to_json_bytes = bass.Bass.to_json_bytes


def _split_multi_waits(self, *args, **kwargs):
    raw = _orig_to_json_bytes(self, *args, **kwargs)
    d = _json.loads(raw)
    changed = False

    def fix_block(o):
        nonlocal changed
        if isinstance(o, dict):
            insts = o.get("instructions")
            if isinstance(insts, list):
                new = []
                for inst in insts:
                    si = inst.get("sync_info") if isinstance(inst, dict) else None
                    waits = (si or {}).get("on_wait") or []
                    if len(waits) > 1:
                        changed = True
                        for i, w in enumerate(waits[:-1]):
                            new.append(
                                {
                                    "name": f"{inst['name']}-sw{i}",
                                    "opcode": "NoOp",
                                    "engine": inst["engine"],
                                    "ins": [],
                                    "outs": [],
                                    "debug": inst.get("debug", 0),
                                    "sync_info": {
                                        "on_wait": [w],
                                        "on_update": [],
                                    },
                                }
                            )
                        si["on_wait"] = [waits[-1]]
                    new.append(inst)
                o["instructions"] = new
            for v in o.values():
                fix_block(v)
        elif isinstance(o, list):
            for v in o:
                fix_block(v)

    fix_block(d)
    if not changed:
        return raw
    return _json.dumps(d).encode()


bass.Bass.to_json_bytes = _split_multi_waits
# ---------------------------------------------------------------------------


def _build_nc() -> bass.Bass:
    nc = bass.Bass()
    # inputs declared float32r: same 4-byte layout as the f32 numpy arrays
    # we feed in; the PE reads them at reduced precision / full speed.
    q = nc.dram_tensor("queryT", [B, D, QLEN], F32R, kind="ExternalInput")
    kv = nc.dram_tensor("keyvalue", [B, S, D], F32R, kind="ExternalInput")
    wqT = nc.dram_tensor("wqT", [D, D], F32R, kind="ExternalInput")
    wkT = nc.dram_tensor("wkT", [D, D], F32R, kind="ExternalInput")
    wvT = nc.dram_tensor("wvT", [D, D], F32R, kind="ExternalInput")
    woT = nc.dram_tensor("woT", [D, D], F32R, kind="ExternalInput")
    poolD = nc.dram_tensor("poolD", [128, 126], F32R, kind="ExternalInput")
    ident = nc.dram_tensor("ident", [128, 128], F32R, kind="ExternalInput")
    ones2 = nc.dram_tensor("ones2", [128, 4, 32], F32R, kind="ExternalInput")
    expand2 = nc.dram_tensor("expand2", [32, 4, 128], F32R, kind="ExternalInput")
    out = nc.dram_tensor("out", [B, QLEN, D], F32, kind="ExternalOutput")

    # DRAM views for partition-major DMA
    q_v = q.rearrange("b (dc p) (ic i) -> b ic p dc i", p=128, ic=ICN)
    kv_v = kv.rearrange("b (t c p) d -> b t p c d", t=KT, c=8, p=128)
    out_v = out.rearrange("b (ic it p) d -> b ic p it d", ic=ICN, it=IT, p=128)
    wq_v = wqT.rearrange("(dc p) o -> p dc o", p=128)
    wk_v = wkT.rearrange("(dc p) o -> p dc o", p=128)
    wv_v = wvT.rearrange("(dc p) o -> p dc o", p=128)
    wo_v = woT.rearrange("(dc p) o -> p dc o", p=128)

    with tile.TileContext(nc) as tc, ExitStack() as ctx:
        # SBUF pools
        singles = ctx.enter_context(tc.tile_pool(name="singles", bufs=1))
        kvp = ctx.enter_context(tc.tile_pool(name="kvp", bufs=2))
        qtp = ctx.enter_context(tc.tile_pool(name="qtp", bufs=2))
        hqp = ctx.enter_context(tc.tile_pool(name="hqp", bufs=2))
        memp = ctx.enter_context(tc.tile_pool(name="memp", bufs=2))
        ep = ctx.enter_context(tc.tile_pool(name="ep", bufs=9))
        attnp = ctx.enter_context(tc.tile_pool(name="attnp", bufs=4))
        rdp = ctx.enter_context(tc.tile_pool(name="rdp", bufs=3))
        bcp = ctx.enter_context(tc.tile_pool(name="bcp", bufs=3))
        vtp = ctx.enter_context(tc.tile_pool(name="vtp", bufs=2))
        outp = ctx.enter_context(tc.tile_pool(name="outp", bufs=2))
        # PSUM pools (8 banks total: 1 + 2 + 5)
        accp = ctx.enter_context(tc.tile_pool(name="accp", bufs=1, space="PSUM"))
        denp = ctx.enter_context(tc.tile_pool(name="denp", bufs=2, space="PSUM"))
        mmp = ctx.enter_context(tc.tile_pool(name="mmp", bufs=5, space="PSUM"))

        # one-time loads
        wq_sb = singles.tile([128, DC, D], F32R)
        nc.sync.dma_start(out=wq_sb, in_=wq_v)
        wk_sb = singles.tile([128, DC, D], F32R)
        nc.sync.dma_start(out=wk_sb, in_=wk_v)
        wv_sb = singles.tile([128, DC, D], F32R)
        nc.sync.dma_start(out=wv_sb, in_=wv_v)
        wo_sb = singles.tile([128, DC, D], F32R)
        nc.sync.dma_start(out=wo_sb, in_=wo_v)
        poolD_sb = singles.tile([128, 126], F32R)
        nc.sync.dma_start(out=poolD_sb, in_=poolD[:, :])
        ident_sb = singles.tile([128, 128], F32R)
        nc.sync.dma_start(out=ident_sb, in_=ident[:, :])
        ones2_sb = singles.tile([128, 4, 32], F32R)
        nc.sync.dma_start(out=ones2_sb, in_=ones2[:, :, :])
        expand2_sb = singles.tile([32, 4, 128], F32R)
        nc.sync.dma_start(out=expand2_sb, in_=expand2[:, :, :])

        for b in range(B):
            # ---- mean-pool keyvalue -> mem [64, 512] -------------------
            pacc = accp.tile([MEM, D], F32, tag="acc")
            for t in range(KT):
                kvt = kvp.tile([128, 8, D], F32R, tag="kv")
                nc.sync.dma_start(out=kvt, in_=kv_v[b, t])
                for c in range(8):
                    sc = 8 * t + c
                    nc.tensor.matmul(
                        pacc,
                        lhsT=poolD_sb[:, 62 - 2 * sc : 126 - 2 * sc],
                        rhs=kvt[:, c, :],
                        start=(sc == 0),
                        stop=(sc == 31),
                    )
            mem_sb = memp.tile([MEM, D], F32R, tag="mem")
            nc.scalar.copy(out=mem_sb, in_=pacc)

            # ---- memT [d, m] (4 chunks of 128 d) -----------------------
            trt = mmp.tile([128, 4, MEM], F32R, tag="mm")
            for c in range(4):
                nc.tensor.transpose(
                    trt[:, c, :],
                    mem_sb[:, 128 * c : 128 * (c + 1)],
                    ident_sb[0:MEM, 0:MEM],
                )
            memT_sb = memp.tile([128, 4, MEM], F32R, tag="memT")
            nc.scalar.copy(out=memT_sb, in_=trt)

            # ---- hkT, stored block-diagonal per head pair --------------
            # hkbd[:, p2, :] is [128, 128]: [0:64, 0:64] = hkT of even head
            # (scaled by 1/8), [64:128, 64:128] = odd head, zero elsewhere.
            hkbd_sb = memp.tile([128, 4, 128], F32R, tag="hkbd")
            nc.scalar.mul(out=hkbd_sb, in_=wk_sb[:, 0, :], mul=0.0)
            for oc in range(4):
                hk_ps = mmp.tile([128, MEM], F32, tag="mm")
                for dc in range(DC):
                    nc.tensor.matmul(
                        hk_ps,
                        lhsT=wk_sb[:, dc, 128 * oc : 128 * (oc + 1)],
                        rhs=memT_sb[:, dc, :],
                        start=(dc == 0),
                        stop=(dc == DC - 1),
                    )
                nc.scalar.mul(
                    out=hkbd_sb[0:64, oc, 0:64], in_=hk_ps[0:64, :], mul=0.125
                )
                nc.scalar.mul(
                    out=hkbd_sb[64:128, oc, 64:128], in_=hk_ps[64:128, :], mul=0.125
                )

            # ---- hv [m, o]; then block-diagonal per head pair ----------
            # hvbd[:, p2, :]: [0:64, 0:64] = V_even [j, dv], [64:128, 64:128]
            # = V_odd, zero elsewhere (odd blocks moved across partitions
            # with a small SBUF->SBUF DMA).
            hv_ps = mmp.tile([MEM, D], F32, tag="mm")
            for dc in range(DC):
                nc.tensor.matmul(
                    hv_ps,
                    lhsT=memT_sb[:, dc, :],
                    rhs=wv_sb[:, dc, :],
                    start=(dc == 0),
                    stop=(dc == DC - 1),
                )
            hv_sb = memp.tile([MEM, D], F32R, tag="hv")
            nc.scalar.copy(out=hv_sb, in_=hv_ps)
            hvbd_sb = memp.tile([128, 4, 128], F32R, tag="hvbd")
            nc.scalar.mul(out=hvbd_sb, in_=wv_sb[:, 0, :], mul=0.0)
            ev = hv_sb.rearrange("m (p2 two dv) -> m p2 two dv", p2=4, two=2)
            nc.scalar.copy(out=hvbd_sb[0:64, :, 0:64], in_=ev[:, :, 0, :])
            nc.sync.dma_start(out=hvbd_sb[64:128, :, 64:128], in_=ev[:, :, 1, :])

            for ic in range(ICN):
                # ---- load qT [d, i] (transposed host-side) -------------
                qT_sb = qtp.tile([128, DC, D], F32R, tag="qT")
                nc.sync.dma_start(out=qT_sb, in_=q_v[b, ic])

                # ---- hqT [o, i] ----------------------------------------
                hqT_sb = hqp.tile([128, DC, D], F32R, tag="hqT")
                for oc in range(DC):
                    hq_ps = mmp.tile([128, D], F32, tag="mm")
                    for dc in range(DC):
                        nc.tensor.matmul(
                            hq_ps,
                            lhsT=wq_sb[:, dc, 128 * oc : 128 * (oc + 1)],
                            rhs=qT_sb[:, dc, :],
                            start=(dc == 0),
                            stop=(dc == DC - 1),
                        )
                    nc.scalar.copy(out=hqT_sb[:, oc, :], in_=hq_ps)

                # ---- attention: scores/exp for all 4 head pairs (one
                # block-diagonal K=128 matmul each), denominators
                # accumulated into one [32, D] tile, one batched
                # reciprocal, then normalize + blockdiag V matmul
                vecT_sb = vtp.tile([128, 4, D], F32R, tag="vecT")
                den_ps = denp.tile([32, D], F32, tag="den")
                e_tiles = []
                for p2 in range(4):
                    sc_ps = mmp.tile([128, D], F32, tag="mm")
                    nc.tensor.matmul(
                        sc_ps,
                        lhsT=hkbd_sb[:, p2, :],
                        rhs=hqT_sb[:, p2, :],
                        start=True,
                        stop=True,
                    )
                    e_sb = ep.tile([128, D], F32R, tag="e")
                    nc.scalar.activation(out=e_sb, in_=sc_ps, func=EXPF)
                    e_tiles.append(e_sb)
                    nc.tensor.matmul(
                        den_ps,
                        lhsT=ones2_sb[:, p2, :],
                        rhs=e_sb,
                        start=(p2 == 0),
                        stop=(p2 == 3),
                    )
                rden_r = rdp.tile([32, D], F32R, tag="rden")
                with nc.allow_low_precision(reason="f32r reciprocal feeds f32r matmul"):
                    nc.vector.reciprocal(out=rden_r, in_=den_ps)
                for p2 in range(4):
                    bc_ps = mmp.tile([128, D], F32, tag="mm")
                    nc.tensor.matmul(
                        bc_ps,
                        lhsT=expand2_sb[:, p2, :],
                        rhs=rden_r,
                        start=True,
                        stop=True,
                    )
                    attn_sb = attnp.tile([128, D], F32R, tag="attn")
                    nc.vector.tensor_mul(attn_sb, e_tiles[p2], bc_ps)
                    uv_ps = mmp.tile([128, D], F32, tag="mm")
                    nc.tensor.matmul(
                        uv_ps,
                        lhsT=hvbd_sb[:, p2, :],
                        rhs=attn_sb,
                        start=True,
                        stop=True,
                    )
                    nc.scalar.copy(out=vecT_sb[:, p2, :], in_=uv_ps)

                # ---- out = vecT.T @ WoT --------------------------------
                out_sb = outp.tile([128, IT, D], F32, tag="o")
                for it in range(IT):
                    o_ps = mmp.tile([128, D], F32, tag="mm")
                    for hc in range(4):
                        nc.tensor.matmul(
                            o_ps,
                            lhsT=vecT_sb[:, hc, 128 * it : 128 * (it + 1)],
                            rhs=wo_sb[:, hc, :],
                            start=(hc == 0),
                            stop=(hc == 3),
                        )
                    nc.vector.tensor_copy(out=out_sb[:, it, :], in_=o_ps)
                # store on the second HWDGE ring (ACT) so loads/stores overlap
                nc.scalar.dma_start(out=out_v[b, ic], in_=out_sb)

    return nc


_NC = None


def _get_nc() -> bass.Bass:
    global _NC
    if _NC is None:
        _NC = _build_nc()
    return _NC


def _consts() -> dict:
    poolD = np.zeros((128, 126), np.float32)
    poolD[0:64, 62] = 1.0 / 64.0
    poolD[64:128, 63] = 1.0 / 64.0
    ident = np.eye(128, dtype=np.float32)
    # den matmul lhsT (per pair p2): accumulate into one [32, D] tile; row
    # 2*p2 sums even-head exp rows (partitions 0-63), row 2*p2+1 sums odd
    # (64-127); rows 8-31 duplicate row 0's pattern so every PSUM row gets
    # a finite positive value (reciprocal runs on the whole tile).
    ones2 = np.zeros((128, 4, 32), np.float32)
    for p2 in range(4):
        ones2[0:64, p2, 2 * p2] = 1.0
        ones2[64:128, p2, 2 * p2 + 1] = 1.0
        if p2 == 0:
            ones2[0:64, p2, 8:32] = 1.0
    # broadcast matmul lhsT, one [32, 128] slice per head pair
    expand2 = np.zeros((32, 4, 128), np.float32)
    for p2 in range(4):
        expand2[2 * p2 + 0, p2, 0:64] = 1.0
        expand2[2 * p2 + 1, p2, 64:128] = 1.0
    return dict(poolD=poolD, ident=ident, ones2=ones2, expand2=expand2)


def run(inputs: dict, trace: bool = False):
    """Run on 8 cores; returns (full_output, BassKernelResults)."""
    query = np.asarray(inputs["query"], np.float32)
    queryT = np.ascontiguousarray(query.transpose(0, 2, 1))
    keyvalue = np.ascontiguousarray(np.asarray(inputs["keyvalue"], np.float32))
    w = {
        "wqT": np.ascontiguousarray(np.asarray(inputs["Wq"], np.float32).T),
        "wkT": np.ascontiguousarray(np.asarray(inputs["Wk"], np.float32).T),
        "wvT": np.ascontiguousarray(np.asarray(inputs["Wv"], np.float32).T),
        "woT": np.ascontiguousarray(np.asarray(inputs["Wo"], np.float32).T),
    }
    consts = _consts()
    nb = query.shape[0]
    per = nb // NCORES
    assert per == B, f"expected {NCORES * B} batches, got {nb}"

    in_maps = []
    for k in range(NCORES):
        m = {
            "queryT": np.ascontiguousarray(queryT[k * per : (k + 1) * per]),
            "keyvalue": np.ascontiguousarray(keyvalue[k * per : (k + 1) * per]),
        }
        m.update(w)
        m.update(consts)
        in_maps.append(m)

    res = run_bass_kernel_spmd(
        _get_nc(), in_maps, core_ids=list(range(NCORES)), trace=trace
    )
    outs = [r["out"] for r in res.results]
    return np.concatenate(outs, axis=0), res


def kernel(**inputs) -> np.ndarray:
    out, _ = run(inputs, trace=False)
    return out
